# revision 23
# baseline (speedup 1.0000x reference)
import sys

sys.path.insert(0, "/opt/trn_rl_repo")

import numpy as np

import concourse.bass as bass
import concourse.tile as tile
import concourse.mybir as mybir
from concourse import bacc
from concourse.bass import ts
from concourse.bass_utils import run_bass_kernel_spmd

N_CORES = 8
C = 32
SIZE = 128
N_FULL = 50000

SCALE_P = 63.5  # (size-1)/2
DELTA_P = 0.0625 * 63.5  # sample spacing in pixel units = 3.96875

NGROUP = 14  # matmul groups: 27k * 2yl * 32c = 1728 rows + bias row + pad

F32 = mybir.dt.float32
F16 = mybir.dt.float16
I32 = mybir.dt.int32

AluOp = mybir.AluOpType
ActFn = mybir.ActivationFunctionType

# x-pair offsets within the gathered 10-voxel span per class
CLASS_OFFS = [(0, 4, 8), (0, 3, 7), (0, 4, 7), (0, 3, 6)]
CLASS_R = [(4, 8), (3, 7), (4, 7), (3, 6)]

_cache = {}
LAST_RES = None


def _emit_idx_group(nc, pools, vg, gl, consts):
    """Vectorized index/weight math for a GROUP of gl tiles (one op chain on
    [128, gl*9] tiles instead of per-tile chains). vg: [128, gl*3] verts view.
    Returns (w9g, w19g, idxeg): weights laid out (k, t, axis) and element-unit
    gather indices laid out (t, ky, kz)."""
    (cpool, gpool, ipool, spool, xpool, zpool, vpool, tpool, dpool, pspool,
     tppool) = pools
    mb_sb, vol, out, c128, ident_sb = consts
    n3 = gl * 3
    n9 = gl * 9

    # p9g[:, k, t*3+axis] = v * 63.5 + bias_k
    p9g = spool.tile([128, 3, n3], F32, tag="p9")
    for k in range(3):
        nc.scalar.activation(
            p9g[:, k, :], vg, ActFn.Copy,
            bias=SCALE_P + (k - 1) * DELTA_P, scale=SCALE_P,
        )
    ci = spool.tile([128, 3 * n3], I32, tag="ci")
    nc.vector.tensor_copy(ci[:], p9g[:].rearrange("p a b -> p (a b)"))
    cf = spool.tile([128, 3 * n3], F32, tag="cf")
    nc.vector.tensor_copy(cf[:], ci[:])
    d9 = spool.tile([128, 3 * n3], F32, tag="d9")
    nc.vector.tensor_tensor(d9[:], p9g[:].rearrange("p a b -> p (a b)"), cf[:],
                            AluOp.subtract)
    m9 = spool.tile([128, 3 * n3], F32, tag="m9")
    nc.vector.tensor_scalar(m9[:], d9[:], 0.0, None, AluOp.is_lt)
    w9g = spool.tile([128, 3 * n3], F32, tag="w9")  # (k, t, axis)
    nc.vector.tensor_tensor(w9g[:], d9[:], m9[:], AluOp.add)
    w19g = spool.tile([128, 3 * n3], F32, tag="w19")  # 1 - w9
    nc.vector.tensor_scalar(w19g[:], w9g[:], -1.0, 1.0, AluOp.mult, AluOp.add)
    i9 = spool.tile([128, 3, gl, 3], F32, tag="i9")  # (k, t, axis)
    nc.vector.tensor_tensor(i9[:].rearrange("p a t b -> p (a t b)"), cf[:],
                            m9[:], AluOp.subtract)

    # row index (z0*128 + y0)*128 + x0(0) per (t, ky, kz), f32-exact (< 2^21)
    rzg = spool.tile([128, 3, gl], F32, tag="rzg")  # (kz, t)
    nc.vector.tensor_scalar(rzg[:], i9[:, :, :, 2], 16384.0, None, AluOp.mult)
    ryxg = spool.tile([128, 3, gl], F32, tag="ryxg")  # (ky, t): y0*128 + x0(0)
    nc.vector.tensor_scalar(ryxg[:], i9[:, :, :, 1], 128.0, None, AluOp.mult)
    xrep = spool.tile([128, 3, gl], F32, tag="xrep")
    for ky in range(3):
        nc.vector.tensor_copy(xrep[:, ky, :], i9[:, 0, :, 0])
    nc.vector.tensor_tensor(ryxg[:], ryxg[:], xrep[:], AluOp.add)
    # materialize (t, ky, kz) layouts and add
    zrep = spool.tile([128, gl, 3, 3], F32, tag="zrep")
    for ky in range(3):
        nc.vector.tensor_copy(
            zrep[:, :, ky, :], rzg[:].rearrange("p kz t -> p t kz")
        )
    yxrep = spool.tile([128, gl, 3, 3], F32, tag="yxrep")
    for kz in range(3):
        nc.vector.tensor_copy(
            yxrep[:, :, :, kz], ryxg[:].rearrange("p ky t -> p t ky")
        )
    idxfg = spool.tile([128, gl * 9], F32, tag="idxfg")  # (t, ky, kz)
    nc.vector.tensor_tensor(
        idxfg[:], zrep[:].rearrange("p a b c -> p (a b c)"),
        yxrep[:].rearrange("p a b c -> p (a b c)"), AluOp.add)
    idxig = ipool.tile([128, n9], I32, tag="idxig")
    nc.vector.tensor_copy(idxig[:], idxfg[:])
    idxeg = ipool.tile([128, n9], I32, tag="idxeg")  # element units (x128)
    nc.vector.tensor_tensor(idxeg[:], idxig[:], c128[:, 0:n9], AluOp.mult)
    return w9g, w19g, idxeg


def _emit_tile2(nc, tc, pools, tl_out_row, offs, consts, st, t_in_g, gl):
    """Gathers + lerp cascade + matmul + output for one tile, using the
    group-vectorized weights/indices."""
    (cpool, gpool, ipool, spool, xpool, zpool, vpool, tpool, dpool, pspool,
     tppool) = pools
    mb_sb, vol, out, c128, ident_sb = consts
    w9g, w19g, idxeg = st
    # column helpers into the (k, t, axis) weight layout
    n3 = gl * 3

    def wcol(w, k, axis):
        col = k * n3 + t_in_g * 3 + axis
        return w[:, col : col + 1]

    # gather: 9 runs of [10 x][2 zl][2 yl][32 c] = 1280 els fp16 per vertex
    G = gpool.tile([128, 9, 1280], F16, tag="G")
    for j in range(9):
        col = t_in_g * 9 + j
        nc.gpsimd.indirect_dma_start(
            out=G[:, j, :],
            out_offset=None,
            in_=vol[:, :],
            in_offset=bass.IndirectOffsetOnAxis(ap=idxeg[:, col : col + 1], axis=1),
        )

    # x-lerp: pairs at span offsets offs[kx] -> X [(ky kz) j][3 kx][128 e]
    # split by ky so each third only needs gathers (3ky..3ky+2), pipelining
    # with the rest of the gather batch
    Gv = G[:].rearrange("p j (x e) -> p j x e", x=10)
    X = xpool.tile([128, 9, 3, 128], F16, tag="X")
    for ky in range(3):
        for kx in range(3):
            A = Gv[:, 3 * ky : 3 * ky + 3, offs[kx], :]
            B = Gv[:, 3 * ky : 3 * ky + 3, offs[kx] + 1, :]
            Xv = X[:, 3 * ky : 3 * ky + 3, kx, :]
            wc = wcol(w9g, kx, 0)
            w1c = wcol(w19g, kx, 0)
            tmp = dpool.tile([128, 3, 128], F16, tag="tx")
            nc.vector.tensor_scalar(Xv, A, w1c, None, AluOp.mult)
            nc.vector.tensor_scalar(tmp[:], B, wc, None, AluOp.mult)
            nc.vector.tensor_tensor(Xv, Xv, tmp[:], AluOp.add)

    # z-lerp: fold zl -> Z [27 k = (ky kz kx)][64 (yl c)]
    Xz = X[:].rearrange("p (ky kz) kx (zl e) -> p ky kz kx zl e", ky=3, zl=2)
    Z = zpool.tile([128, 27, 64], F16, tag="Z")
    Zz = Z[:].rearrange("p (ky kz kx) e -> p ky kz kx e", ky=3, kz=3)
    for kz in range(3):
        A = Xz[:, :, kz, :, 0, :]
        B = Xz[:, :, kz, :, 1, :]
        Zv = Zz[:, :, kz, :, :]
        wc = wcol(w9g, kz, 2)
        w1c = wcol(w19g, kz, 2)
        tmp = dpool.tile([128, 3, 3, 64], F16, tag="tz")
        nc.vector.tensor_scalar(Zv, A, w1c, None, AluOp.mult)
        nc.vector.tensor_scalar(tmp[:], B, wc, None, AluOp.mult)
        nc.vector.tensor_tensor(Zv, Zv, tmp[:], AluOp.add)

    # y weights folded into matmul rows: V[(ky kz kx yl c)] = wy * Z
    V = vpool.tile([128, NGROUP * 128], F16, tag="V")
    Vv = V[:, 0:1728].rearrange("p (ky a yl c) -> p ky a yl c", ky=3, yl=2, c=32)
    Zy = Z[:].rearrange("p (ky a) (yl c) -> p ky a yl c", ky=3, yl=2)
    for ky in range(3):
        for yl in range(2):
            wc = wcol(w9g if yl else w19g, ky, 1)
            nc.vector.tensor_scalar(
                Vv[:, ky, :, yl, :], Zy[:, ky, :, yl, :], wc, None, AluOp.mult
            )
    nc.vector.memset(V[:, 1728:1729], 1.0)  # bias row
    nc.vector.memset(V[:, 1729:], 0.0)

    # transpose V via PE identity matmuls (PSUM) + Act copies back to SBUF
    VT = tpool.tile([128, NGROUP, 128], F16, tag="VT")
    for t in range(NGROUP):
        tp = tppool.tile([128, 128], F32, tag="tp")
        nc.tensor.matmul(tp[:], V[:, ts(t, 128)], ident_sb[:], start=True, stop=True)
        nc.scalar.activation(VT[:, t, :], tp[:], ActFn.Copy, bias=0.0, scale=1.0)

    psum = pspool.tile([128, C], F32, tag="ps")
    for t in range(NGROUP):
        nc.tensor.matmul(
            psum[:], VT[:, t, :], mb_sb[:, ts(t, C)],
            start=(t == 0), stop=(t == NGROUP - 1),
        )
    osb = ipool.tile([128, C], F32, tag="osb")
    nc.scalar.activation(osb[:], psum[:], ActFn.Copy, bias=0.0, scale=1.0)
    nc.sync.dma_start(out[ts(tl_out_row, 128), :], osb[:])


def _build(tile_counts):
    """tile_counts: tuple of per-class 128-vertex tile counts."""
    tiles = sum(tile_counts)
    nv = tiles * 128
    nc = bacc.Bacc("TRN2", target_bir_lowering=False, debug=False)

    vol = nc.dram_tensor(
        "vol", [131072, 2048], F16, kind="ExternalInput"
    ).ap()  # flat [z y x][zl yl c] els; 2-D big-inner so modeled descs are 2560B
    verts = nc.dram_tensor("verts", [nv, 3], F32, kind="ExternalInput").ap()
    mbig = nc.dram_tensor("mbig", [128, NGROUP * C], F16, kind="ExternalInput").ap()
    identd = nc.dram_tensor("ident", [128, 128], F16, kind="ExternalInput").ap()
    out = nc.dram_tensor("out", [nv, C], F32, kind="ExternalOutput").ap()

    with tile.TileContext(nc) as tc:
        with (
            tc.tile_pool(name="const", bufs=1) as cpool,
            tc.tile_pool(name="gather", bufs=6) as gpool,
            tc.tile_pool(name="idx", bufs=3) as ipool,
            tc.tile_pool(name="small", bufs=3) as spool,
            tc.tile_pool(name="xl", bufs=2) as xpool,
            tc.tile_pool(name="zl", bufs=2) as zpool,
            tc.tile_pool(name="vp", bufs=3) as vpool,
            tc.tile_pool(name="vt", bufs=3) as tpool,
            tc.tile_pool(name="dd", bufs=4) as dpool,
            tc.tile_pool(name="psum", bufs=4, space="PSUM") as pspool,
            tc.tile_pool(name="tpsum", bufs=4, space="PSUM") as tppool,
        ):
            pools = (cpool, gpool, ipool, spool, xpool, zpool, vpool, tpool,
                     dpool, pspool, tppool)
            mb_sb = cpool.tile([128, NGROUP * C], F16, tag="mb")
            nc.sync.dma_start(mb_sb[:], mbig[:])
            vall = cpool.tile([128, tiles * 3], F32, tag="vall")
            nc.sync.dma_start(vall[:], verts.rearrange("(t p) a -> p t a", p=128))
            c128 = cpool.tile([128, 72], I32, tag="c128")
            nc.vector.memset(c128[:], 128)
            ident_sb = cpool.tile([128, 128], F16, tag="ident")
            nc.sync.dma_start(ident_sb[:], identd[:])
            consts = (mb_sb, vol, out, c128, ident_sb)

            sched = []
            for cls, n_t in enumerate(tile_counts):
                sched.extend([CLASS_OFFS[cls]] * n_t)
            GLEN = 8  # tiles per vectorized index group
            for g0 in range(0, len(sched), GLEN):
                gl = min(GLEN, len(sched) - g0)
                st = _emit_idx_group(
                    nc, pools, vall[:, g0 * 3 : (g0 + gl) * 3], gl, consts)
                for t_in_g in range(gl):
                    tl = g0 + t_in_g
                    _emit_tile2(nc, tc, pools, tl, sched[tl], consts, st, t_in_g, gl)

    nc.compile()
    return nc


def _get_nc(tile_counts):
    key = tuple(tile_counts)
    if key not in _cache:
        _cache[key] = _build(key)
    return _cache[key]


def _host_prep(voxel_features, vertices, w_d1, b_d1, w_d2, b_d2, w_c1, b_c1, w_c2,
               b_c2, conv_w, conv_b):
    # volume -> [z, y, x, zl, yl, c] fp16 (x4 redundant corner-pair layout)
    v = np.transpose(np.asarray(voxel_features, np.float32)[0], (1, 2, 3, 0))
    v = np.ascontiguousarray(v).astype(np.float16)  # [z, y, x, c]
    vp = np.empty((SIZE + 1, SIZE + 1, SIZE, C), np.float16)
    vp[:SIZE, :SIZE] = v
    vp[SIZE, :SIZE] = v[SIZE - 1]
    vp[:SIZE, SIZE] = vp[:SIZE, SIZE - 1]
    vp[SIZE, SIZE] = vp[SIZE, SIZE - 1]
    vol4 = np.empty((SIZE, SIZE, SIZE, 2, 2, C), np.float16)
    for zl in range(2):
        for yl in range(2):
            vol4[:, :, :, zl, yl, :] = vp[zl : zl + SIZE, yl : yl + SIZE]
    vol4 = vol4.reshape(131072, 2048)

    f8 = np.float64
    Wd = np.asarray(w_d2, f8) @ np.asarray(w_d1, f8)
    bd = np.asarray(b_d1, f8) @ np.asarray(w_d2, f8).T + np.asarray(b_d2, f8)
    Wc = np.asarray(w_c2, f8) @ np.asarray(w_c1, f8)
    bc = np.asarray(b_c1, f8) @ np.asarray(w_c2, f8).T + np.asarray(b_c2, f8)
    cw = np.asarray(conv_w, f8)[:, :, 0, :]  # [o, c', k]

    A = np.einsum("ock,cd->odk", cw, Wd)  # [o, c, k]
    M = np.moveaxis(A, 2, 0).copy()  # [k, o, c], ref order k = kx*9 + ky*3 + kz
    M[13] += Wc - A.sum(axis=2)
    bias_tot = cw.sum(axis=2) @ bd + np.asarray(conv_b, f8) + bc

    # Mbig row r = ((ky*3 + kz)*3 + kx)*64 + yl*32 + c -> M[kx*9+ky*3+kz][o, c]
    Mbig = np.zeros((NGROUP * 128, C), np.float64)
    for ky in range(3):
        for kz in range(3):
            for kx in range(3):
                r0 = ((ky * 3 + kz) * 3 + kx) * 64
                m = M[kx * 9 + ky * 3 + kz].T  # [c, o]
                Mbig[r0 : r0 + 32] = m
                Mbig[r0 + 32 : r0 + 64] = m
    Mbig[1728] = bias_tot
    mb_host = np.ascontiguousarray(
        Mbig.reshape(NGROUP, 128, C).transpose(1, 0, 2).reshape(128, NGROUP * C)
    ).astype(np.float16)
    return vol4, mb_host


def _classify(vp):
    """vp: [n, 3] f32 vertices -> class id, replicating the device's f32
    arithmetic exactly (p = fl32(v*63.5) + bias_k, floors in f32)."""
    q = vp[:, 0].astype(np.float32) * np.float32(SCALE_P)
    x0 = np.floor(q + np.float32(SCALE_P - DELTA_P)).astype(np.int64)
    x1 = np.floor(q + np.float32(SCALE_P)).astype(np.int64)
    x2 = np.floor(q + np.float32(SCALE_P + DELTA_P)).astype(np.int64)
    r1 = x1 - x0
    r2 = x2 - x0
    cls = np.full(vp.shape[0], -1, np.int64)
    for i, (a, b) in enumerate(CLASS_R):
        cls[(r1 == a) & (r2 == b)] = i
    assert (cls >= 0).all(), "unexpected x-spacing class"
    return cls


def kernel(**inputs):
    vol4, mb_host = _host_prep(**inputs)
    vp = np.asarray(inputs["vertices"], np.float32)[0]
    n = vp.shape[0]

    # shard vertices round-robin-contiguous, then class-sort within each core
    per_core = (n + N_CORES - 1) // N_CORES
    in_maps = []
    counts_ref = None
    for i in range(N_CORES):
        seg = vp[i * per_core : min((i + 1) * per_core, n)]
        cls = _classify(seg)
        order = np.argsort(cls, kind="stable")
        seg_sorted = seg[order]
        cls_sorted = cls[order]
        tile_counts = []
        v_parts = []
        for c in range(len(CLASS_OFFS)):
            part = seg_sorted[cls_sorted == c]
            n_t = (len(part) + 127) // 128
            if len(part) < n_t * 128:
                fill = part[:1] if len(part) else None
                pad = np.repeat(fill, n_t * 128 - len(part), axis=0) if fill is not None else None
                part = np.concatenate([part, pad], axis=0) if pad is not None else part
            tile_counts.append(n_t)
            v_parts.append(part)
        verts_padded = np.concatenate(
            [p for p in v_parts if len(p)], axis=0
        ).astype(np.float32)
        if counts_ref is None:
            counts_ref = tuple(tile_counts)
        else:
            # all cores must share one compiled program: equalize tile counts
            counts_ref = tuple(max(a, b) for a, b in zip(counts_ref, tile_counts))
        in_maps.append({"verts": verts_padded, "tile_counts": tuple(tile_counts),
                        "order": order, "seg_len": len(seg)})

    # pad every core's segments up to the common per-class tile counts
    for i in range(N_CORES):
        m = in_maps[i]
        tc_i = m["tile_counts"]
        v = m["verts"]
        pieces = []
        start = 0
        for c in range(len(CLASS_OFFS)):
            seg_c = v[start : start + tc_i[c] * 128]
            start += tc_i[c] * 128
            need = counts_ref[c] * 128
            if len(seg_c) < need:
                fill = seg_c[:1] if len(seg_c) else v[:1]
                seg_c = np.concatenate(
                    [seg_c, np.repeat(fill, need - len(seg_c), axis=0)], axis=0
                )
            pieces.append(seg_c)
        m["verts"] = np.ascontiguousarray(np.concatenate(pieces, axis=0))

    nc = _get_nc(counts_ref)
    run_maps = [
        {"vol": vol4, "verts": in_maps[i]["verts"], "mbig": mb_host,
         "ident": np.eye(128, dtype=np.float16)}
        for i in range(N_CORES)
    ]
    res = run_bass_kernel_spmd(nc, run_maps, list(range(N_CORES)))
    global LAST_RES
    LAST_RES = res

    out = np.empty((n, C), np.float32)
    bounds = np.cumsum([0] + [c * 128 for c in counts_ref])
    for i in range(N_CORES):
        seg_len = in_maps[i]["seg_len"]
        raw = res.results[i]["out"]
        order = in_maps[i]["order"]
        seg = vp[i * per_core : min((i + 1) * per_core, n)]
        c_of = _classify(seg)[order]
        vals = []
        for c in range(len(CLASS_OFFS)):
            k = int((c_of == c).sum())
            vals.append(raw[bounds[c] : bounds[c] + k])
        sorted_out = np.concatenate(vals, axis=0)
        seg_out = np.empty_like(sorted_out)
        seg_out[order] = sorted_out
        out[i * per_core : i * per_core + seg_len] = seg_out
    return out.reshape(1, n, C).astype(np.float32)
